# revision 27
# baseline (speedup 1.0000x reference)
"""Trainium2 kernel for per-subject linear heads (moe_routing).

Computes out[i] = x[i] @ W[subject_ids[i]] + b[subject_ids[i]] for
B=256, D=2048, S=8 subjects, OUT=1000.

Sharding: expert-parallel — core s owns subject s. Each core reads only
its own (2048, 1000) weight slice from HBM, so the total weight traffic
across the chip is W read exactly once (vs 8x for batch-data-parallel
with a replicated table). Samples are grouped by subject on the host,
padded to a fixed capacity C, and fed to an SPMD Bass/Tile kernel;
outputs are scattered back to the original order.

The kernel is HBM-bandwidth-bound (per-core cap ~358 GB/s), so weights
and activations are cast to bf16 on the host: W traffic halves to
4.1 MB/core and the PE runs single-pass matmuls (vs 4-pass fp32) that
never gate the DMA stream. bf16 dot over K=2048 costs ~1.6e-3 rel err.

Kernel-side notes:
- The bias is folded into the matmul accumulation as a rank-1 update
  (ones row carried as an extra k-slot of x, times the [1, OUT] bias).
- This walrus build rejects any instruction with more than one sync
  wait, so the kernel is structured so no instruction ever needs two:
  a scratch-fed warm-up matmul absorbs the x-DMA wait, and the 8 DMAs
  (x, bias, 4 W chunks, 2 y writes) map 1:1 onto the 8 HWDGE
  completion-sem lanes, so no lane-reuse wait ever appears.
- W is pre-permuted on the host so each chunk DMA reads one contiguous
  8 KB run per partition.
- A chain of throwaway matmuls on a memset scratch tile keeps the PE
  busy from kernel start so the HAM clock-gate reaches 2.4 GHz before
  the real matmul stream begins.
"""

import numpy as np
import ml_dtypes

import concourse.bass as bass
import concourse.mybir as mybir
import concourse.tile as tile
from concourse.bass_utils import run_bass_kernel_spmd
from concourse.vector_clock import ScopedClock, VectorClock

B = 256
D = 2048
S = 8
OUT = 1000
P = 128
KO = D // P          # 16 k-tiles of 128
NT = 500             # psum n-tile (<= 512 fp32 / bank), 2 tiles cover OUT

# W chunk k-tile ranges and their HWDGE ring (True = SP/sync ring).
# Four uniform 1MB chunks with 8KB-per-partition descriptors: smaller /
# more numerous chunk DMAs measurably let the slow SDMA engine (15)
# fall behind and stall the stream tail, so keep the descriptor shape
# of the best-measured configuration.
CHUNKS = [(0, 4), (4, 8), (8, 12), (12, 16)]
CHUNK_ON_SYNC = [True, False, True, False]
N_CHUNKS = len(CHUNKS)

SPINS_PRE = 16       # PE warm-up matmuls before the real stream
SPIN_N = 128         # spin matmul free dim (short, so cut-over is fast)
SPINS_GAP = 0        # filler matmuls after EVERY chunk. Measured
                     # NET-NEGATIVE here: the spins serialize after each
                     # chunk's matmuls and the PE (still ramping) falls
                     # behind the DMA stream by more than the cold-clock
                     # tail they were meant to avoid.
SPINS_TAIL = 14      # spin block after the second-to-last chunk's
                     # matmuls only: fills the ~1.8us idle gap so the
                     # HAM clock is at speed when the last chunk lands
                     # (a cold tail burst costs ~2.0us vs ~0.9us hot).

TRACE = False        # set by test harness to collect an NTFF profile
LAST_RESULTS = None  # BassKernelResults of the most recent run

_nc_cache = {}


class _FastExitTileContext(tile.TileContext):
    """TileContext with a single-wait-per-instruction, barrier-free exit.

    This walrus build rejects instructions with >1 sync wait, and the
    stock exit (one Drain waiting on every semaphore + two all-engine
    EVSEM-butterfly barriers) both violates that and costs ~8 us. Here
    SP emits one drain per logical processor (each <=1 wait), then
    hands off to GpSimd via a fresh semaphore; GpSimd resets the DMA
    queues and clears all Tile semaphores (required so a re-execution
    of the NEFF starts from zeroed sems). By the time SP's drains have
    observed every semaphore at its final value, every engine has
    retired its last instruction, so the butterfly barriers are
    unnecessary.
    """

    def _drain_and_barrier(self, tick_clock, wait_clock):
        nc = self.nc
        gc = tick_clock.global_clock
        n = len(gc)
        last = None
        for i in range(n):
            if gc[i] <= 0:
                continue
            vec = [0] * n
            vec[i] = gc[i]
            d = nc.sync.drain()
            wait_clock.add_sem_waits(d.ins, ScopedClock({None: VectorClock(vec)}))
            last = d

        assert self.sems is not None
        popped = nc._tile_sem_poison_stack.pop()
        assert popped is self._sem_poison
        sems = list(self.sems.allocated().values())
        if last is not None:
            handoff = nc.alloc_semaphore(name="exit_handoff")
            last.then_inc(handoff, 1)
            nc.gpsimd.wait_ge(handoff, 1)
            nc.clear_and_free_semaphores(sems)
            nc.gpsimd.sem_clear(handoff)
            nc.release_semaphore(handoff)
        else:
            nc.clear_and_free_semaphores(sems)


def _build(C):
    """Per-core program: y[C, OUT] = xT.T @ w + bias.

    xT   : [P, KO+1, C]  bf16   xT[p, ko, c] = x_subject[c, ko*P + p]
                                for ko < KO; last slot all-ones (bias).
    w    : [P, KO*OUT]   bf16   host-permuted weights;
                                w[p, ko*OUT+n] = W[ko*P + p, n].
    bias : [1, OUT]      bf16   the subject's bias row.
    y    : [C, OUT]      bf16   output (upcast to fp32 on the host).
    """
    cdt = mybir.dt.bfloat16
    nc = bass.Bass(enable_partition_id=False)
    xT = nc.dram_tensor("xT", [P, KO + 1, C], cdt, kind="ExternalInput")
    w = nc.dram_tensor("w", [P, KO * OUT], cdt, kind="ExternalInput")
    bias = nc.dram_tensor("bias", [1, OUT], cdt, kind="ExternalInput")
    y = nc.dram_tensor("y", [C, OUT], cdt, kind="ExternalOutput")

    m_tiles = [(m0, min(P, C - m0)) for m0 in range(0, C, P)]

    with _FastExitTileContext(nc) as tc:
        with (
            tc.tile_pool(name="wpool", bufs=N_CHUNKS) as wpool,
            tc.tile_pool(name="xpool", bufs=1) as xpool,
            tc.tile_pool(name="bpool", bufs=1) as bpool,
            tc.tile_pool(name="spool", bufs=1) as spool,
            tc.tile_pool(name="opool", bufs=4) as opool,
            tc.tile_pool(name="psum", bufs=1, space="PSUM") as psum_pool,
        ):
            # PE warm-up scratch: memset by GpSimd so the first spin
            # matmul's only wait is the GpSimd semaphore.
            scratch = spool.tile([P, SPIN_N], cdt)
            nc.gpsimd.memset(scratch[:], 0.0)

            # Exactly 8 HWDGE DMAs on the 8 completion-sem lanes: x(0),
            # bias(1), chunks(2..5), y0(6), y1(7) — no lane reuse, no
            # SWDGE traffic at all (SWDGE descriptor-ring writes share
            # AXI ports with SDMA engines 7/15 and stall the stream).
            x_tile = xpool.tile([P, KO + 1, C], cdt)
            nc.scalar.dma_start(x_tile[:], xT[:])
            b_tile = bpool.tile([1, OUT], cdt)
            nc.scalar.dma_start(b_tile[:], bias[:])

            w_tiles = []
            for (k0, k1), on_sync in zip(CHUNKS, CHUNK_ON_SYNC):
                wt = wpool.tile([P, (k1 - k0) * OUT], cdt)
                eng = nc.sync if on_sync else nc.scalar
                eng.dma_start(wt[:], w[:, k0 * OUT : k1 * OUT])
                w_tiles.append(wt)

            # For mc <= 64 the two n-tiles share one PSUM bank on
            # disjoint column halves of the PE array (tile_position), so
            # their matmul streams run concurrently on independent
            # 32x32 sub-arrays.
            col_tiled = all(mc <= 64 for _, mc in m_tiles)
            psums = {}
            tilepos = {}
            for mi, (m0, mc) in enumerate(m_tiles):
                if col_tiled:
                    joint = psum_pool.tile(
                        [P, NT], mybir.dt.float32, name=f"psum_{mi}"
                    )
                    psums[f"joint_{mi}"] = joint
                    psums[(mi, 0)] = joint[0:mc]
                    psums[(mi, 1)] = joint[64 : 64 + mc]
                    tilepos[(mi, 0)] = (0, 0)
                    tilepos[(mi, 1)] = (0, 64)
                else:
                    for n in range(2):
                        psums[(mi, n)] = psum_pool.tile(
                            [mc, NT], mybir.dt.float32, name=f"psum_{mi}_{n}"
                        )
                        tilepos[(mi, n)] = None
            spin_ps = psum_pool.tile([1, SPIN_N], mybir.dt.float32, name="spin_ps")

            def spin(k):
                for _ in range(k):
                    nc.tensor.matmul(
                        spin_ps[:, :],
                        scratch[:, 0:1],
                        scratch[:, :SPIN_N],
                        start=True,
                        stop=True,
                    )

            spin(SPINS_PRE)
            # Absorbs the x-DMA wait (scratch has no DMA dependency), so
            # later matmuls each need only their own chunk/bias wait.
            warm = psum_pool.tile([1, C], mybir.dt.float32, name="warm")
            nc.tensor.matmul(
                warm[:, :],
                scratch[:, 0:1],
                x_tile[:, 0, :],
                start=True,
                stop=True,
            )
            # Open each accumulation group with the rank-1 bias update:
            # ones[1, mc].T @ bias[1, NT].
            for mi, (m0, mc) in enumerate(m_tiles):
                for n in range(2):
                    nc.tensor.matmul(
                        psums[(mi, n)][:, :],
                        x_tile[0:1, KO, m0 : m0 + mc],
                        b_tile[0:1, n * NT : (n + 1) * NT],
                        start=True,
                        stop=False,
                        tile_position=tilepos[(mi, n)],
                    )
            # k-contiguous loop: each W chunk is consumed for every
            # (m, n) output tile as soon as it lands, then is dead.
            for ci, (k0, k1) in enumerate(CHUNKS):
                wt = w_tiles[ci]
                for ko in range(k0, k1):
                    base = (ko - k0) * OUT
                    for mi, (m0, mc) in enumerate(m_tiles):
                        lhsT = x_tile[:, ko, m0 : m0 + mc]
                        for n in range(2):
                            nc.tensor.matmul(
                                psums[(mi, n)][:, :],
                                lhsT,
                                wt[:, base + n * NT : base + (n + 1) * NT],
                                start=False,
                                stop=(ko == KO - 1),
                                tile_position=tilepos[(mi, n)],
                            )
                # No spins after the last chunk: its k-tiles must run
                # the moment it lands.
                if SPINS_GAP and ci < N_CHUNKS - 1:
                    spin(SPINS_GAP)
                if SPINS_TAIL and ci == N_CHUNKS - 2:
                    spin(SPINS_TAIL)
            # Drain: one joint PSUM -> SBUF copy (fp32 -> bf16 cast on
            # DVE covers both n-halves when they share a bank), then
            # both y writes issue in parallel on sync + scalar HWDGE —
            # each a single copy-sem wait.
            for mi, (m0, mc) in enumerate(m_tiles):
                if col_tiled:
                    ot = opool.tile([P, NT], cdt)
                    nc.vector.tensor_copy(ot[:], psums[f"joint_{mi}"][:])
                    nc.sync.dma_start(y[m0 : m0 + mc, 0:NT], ot[0:mc])
                    nc.scalar.dma_start(
                        y[m0 : m0 + mc, NT : 2 * NT], ot[64 : 64 + mc]
                    )
                else:
                    for n in range(2):
                        ot = opool.tile([mc, NT], cdt)
                        nc.vector.tensor_copy(ot[:], psums[(mi, n)][:])
                        deng = nc.sync if n == 0 else nc.scalar
                        deng.dma_start(y[m0 : m0 + mc, n * NT : (n + 1) * NT], ot[:])
    return nc


def _capacity(max_count):
    c = 48
    while c < max_count:
        c *= 2
    return c


def kernel(x, subject_ids, W, b):
    global LAST_RESULTS
    bf16 = ml_dtypes.bfloat16
    x = np.asarray(x, dtype=np.float32)
    sid = np.asarray(subject_ids).astype(np.int64)
    W = np.asarray(W, dtype=np.float32)
    b = np.asarray(b, dtype=np.float32)

    groups = [np.nonzero(sid == s)[0] for s in range(S)]
    C = _capacity(max((len(g) for g in groups), default=1))

    key = (C, SPINS_PRE, SPINS_GAP)
    if key not in _nc_cache:
        _nc_cache[key] = _build(C)
    nc = _nc_cache[key]

    # [p, ko*OUT + n] = W[s, ko*P + p, n]: each chunk DMA reads one
    # contiguous (k1-k0)*2KB run per partition.
    W_perm = np.ascontiguousarray(
        W.astype(bf16).reshape(S, KO, P, OUT).transpose(0, 2, 1, 3)
    ).reshape(S, P, KO * OUT)
    b16 = b.astype(bf16)

    in_maps = []
    for s in range(S):
        idx = groups[s]
        xs = np.zeros((C, D), dtype=np.float32)
        xs[: len(idx)] = x[idx]
        # [p, ko, c] = xs[c, ko*P + p]; extra all-ones k-slot for bias
        xT = np.empty((P, KO + 1, C), dtype=bf16)
        xT[:, :KO, :] = xs.T.reshape(KO, P, C).transpose(1, 0, 2).astype(bf16)
        xT[:, KO, :] = 1.0
        in_maps.append({"xT": xT, "w": W_perm[s], "bias": b16[s : s + 1]})

    LAST_RESULTS = run_bass_kernel_spmd(
        nc, in_maps, core_ids=list(range(S)), trace=TRACE
    )

    out = np.zeros((B, OUT), dtype=np.float32)
    for s in range(S):
        idx = groups[s]
        out[idx] = LAST_RESULTS.results[s]["y"][: len(idx)].astype(np.float32)
    return out
